# revision 8
# baseline (speedup 1.0000x reference)
"""Trainium2 Bass kernel for MultiHeadAttention (B=4, S=2048, D=1024, H=16).

Sharding (8 cores): core c = (batch b=c//2, head-group g=c%2).
Each core handles 1 batch x 8 heads (proj dims g*512..(g+1)*512).

Per-core device program (all matmuls fp32r):
  - Projections from host-pre-transposed inputs:
      QT[pd, tok] = wq_t.T @ xq_t      (pd = 512 proj dims, tok = 2048)
      KT[pd, tok] = wk_t.T @ xk_t
      V[tok, vd]  = xv_t.T @ wv_t      (token-major, augmented w/ ones col/head)
  - Attention per head h (depth 64), scores computed TRANSPOSED:
      S'[k, q] = KT_h.T @ QT_h ;  E = exp(S'/8)  (softmax max-sub skipped: |S| small)
      rawT[d', q] = V_aug_h.T @ E   -> row 64 is the softmax denominator
      OT[d', q] = rawT[:64] * (1/denom)  (recip + partition-broadcast + DVE mul)
  - Dense partial: out[tok, n] = OT.T @ ds_t  (ds_t = dense_w[:, g-slice].T)
Host: out[b] = partial[2b] + partial[2b+1] + dense_b.

Self-contained: hardcodes shapes; builds/compiles the Bass program once per
process and reuses it.
"""

import numpy as np
from contextlib import ExitStack

import concourse.bass as bass
import concourse.tile as tile
from concourse import bacc, mybir
from concourse.bass_utils import run_bass_kernel_spmd

F32 = mybir.dt.float32
F32R = mybir.dt.float32r
EXP = mybir.ActivationFunctionType.Exp

P = 128
S = 2048          # tokens per batch
DM = 1024         # d_model
PD = 512          # proj dims per core (8 heads x 64)
NDC = DM // P     # 8 d_model chunks
NPT = PD // P     # 4 proj partition tiles
NTQ = 4           # token quarters (512)
NTT = 16          # token tiles (128)
NH = 8            # heads per core
DEP = 64          # head depth
VW = NH * (DEP + 1)   # V tile width with ones-augmentation (520)


def _r(ap):
    return ap.bitcast(F32R)


def _emit(nc, tc, ctx, d, with_bias):
    has_qb, has_kb, has_vb = with_bias

    res = ctx.enter_context(tc.tile_pool(name="res", bufs=1))
    xw = ctx.enter_context(tc.tile_pool(name="xw", bufs=18))
    wpool = ctx.enter_context(tc.tile_pool(name="w", bufs=1))
    espool = ctx.enter_context(tc.tile_pool(name="es", bufs=3))
    bcpool = ctx.enter_context(tc.tile_pool(name="bc", bufs=2))
    rcpool = ctx.enter_context(tc.tile_pool(name="rc", bufs=2))
    outpool = ctx.enter_context(tc.tile_pool(name="osb", bufs=2))
    ps = ctx.enter_context(tc.tile_pool(name="ps", bufs=2, space="PSUM"))
    pvps = ctx.enter_context(tc.tile_pool(name="pvps", bufs=2, space="PSUM"))

    # ---- resident tiles -------------------------------------------------
    QT = [res.tile([P, S], F32R, name=f"QT{pt}", tag=f"QT{pt}") for pt in range(NPT)]
    KT = [res.tile([P, S], F32R, name=f"KT{pt}", tag=f"KT{pt}") for pt in range(NPT)]
    V = [res.tile([P, VW], F32R, name=f"V{tt}", tag=f"V{tt}") for tt in range(NTT)]

    # ones columns of V (col h*65+64 = 1.0); DMA'd from a DRAM constant
    # (DVE memset can't produce float32r)
    for tt in range(NTT):
        v3 = V[tt].rearrange("p (h c) -> p h c", c=DEP + 1)
        nc.sync.dma_start(out=v3[:, :, DEP : DEP + 1], in_=_r(d["ones_c"][:, :]))

    bias_sb = {}
    if has_qb or has_kb or has_vb:
        ones_row = res.tile([1, 512], F32R, name="ones_row", tag="ones_row")
        nc.sync.dma_start(out=ones_row, in_=_r(d["ones_r"][:, :]))
        for flag, nm in ((has_qb, "bq"), (has_kb, "bk"), (has_vb, "bv")):
            if flag:
                bias_sb[nm] = res.tile([1, PD], F32R, name=f"{nm}_sb", tag=f"{nm}_sb")
                nc.sync.dma_start(out=bias_sb[nm], in_=_r(d[nm][:, :]))

    # ---- V projection ---------------------------------------------------
    wv_ch = []
    for dc in range(NDC):
        wt = wpool.tile([P, PD], F32R, name=f"wv{dc}", tag=f"w{dc}")
        nc.sync.dma_start(out=wt, in_=_r(d["wv_t"][dc * P : (dc + 1) * P, :]))
        wv_ch.append(wt)

    for tq in range(NTQ):
        xv_ch = []
        for dc in range(NDC):
            xt = xw.tile([P, 512], F32R, name=f"xv{tq}_{dc}", tag="xt")
            nc.sync.dma_start(
                out=xt, in_=_r(d["xv_t"][dc * P : (dc + 1) * P, tq * 512 : (tq + 1) * 512])
            )
            xv_ch.append(xt)
        for tb in range(4):
            tt = tq * 4 + tb
            pst = ps.tile([P, 512], F32, name=f"psv{tt}", tag="proj")
            for dc in range(NDC):
                nc.tensor.matmul(
                    pst,
                    _r(xv_ch[dc][:, tb * P : (tb + 1) * P]),
                    _r(wv_ch[dc]),
                    start=(dc == 0),
                    stop=(dc == NDC - 1 and not has_vb),
                )
            if has_vb:
                nc.tensor.matmul(
                    pst, _r(ones_row[:, :P]), _r(bias_sb["bv"]), start=False, stop=True
                )
            v3 = V[tt].rearrange("p (h c) -> p h c", c=DEP + 1)
            nc.vector.tensor_copy(v3[:, :, 0:DEP], pst)

    # ---- K/Q projections + attention ------------------------------------
    def proj_stage(wname, xname, bname, out_tiles, wtag0, tqs, w_ch):
        """Project out_tiles[:, tq*512...] for tq in tqs. w_ch cached across calls."""
        if not w_ch:
            for dc in range(NDC):
                wt = wpool.tile(
                    [P, PD], F32R, name=f"{wname}_{dc}", tag=f"w{wtag0 + dc}"
                )
                nc.sync.dma_start(out=wt, in_=_r(d[wname][dc * P : (dc + 1) * P, :]))
                w_ch.append(wt)
        has_b = bname in bias_sb
        for tq in tqs:
            x_ch = []
            for dc in range(NDC):
                xt = xw.tile([P, 512], F32R, name=f"x{xname}{tq}_{dc}", tag="xt")
                nc.sync.dma_start(
                    out=xt,
                    in_=_r(d[xname][dc * P : (dc + 1) * P, tq * 512 : (tq + 1) * 512]),
                )
                x_ch.append(xt)
            for pt in range(NPT):
                pst = ps.tile([P, 512], F32, name=f"psp{xname}{tq}_{pt}", tag="proj")
                for dc in range(NDC):
                    nc.tensor.matmul(
                        pst,
                        _r(w_ch[dc][:, pt * P : (pt + 1) * P]),
                        _r(x_ch[dc]),
                        start=(dc == 0),
                        stop=(dc == NDC - 1 and not has_b),
                    )
                if has_b:
                    nc.tensor.matmul(
                        pst,
                        _r(bias_sb[bname][:, pt * P : (pt + 1) * P]),
                        _r(ones_row),
                        start=False,
                        stop=True,
                    )
                nc.vector.tensor_copy(
                    out_tiles[pt][:, tq * 512 : (tq + 1) * 512], pst
                )

    # K fully projected up front (attention needs all key tokens)
    proj_stage("wk_t", "xk_t", "bk", KT, 8, range(NTQ), [])

    OT_all = {}
    wq_ch = []
    for qb in range(2):
        # queries only for this half
        proj_stage("wq_t", "xq_t", "bq", QT, 0, (qb * 2, qb * 2 + 1), wq_ch)

        # ---- attention for this token half (query cols qb*1024..) ------
        for h in range(NH):
            pt, half = h // 2, h % 2
            r0 = half * DEP
            pv = [
                pvps.tile([DEP + 1, 512], F32, name=f"pv{qb}_{h}_{qs}", tag="pv")
                for qs in range(2)
            ]
            for kt in range(NTT):
                qk = ps.tile([P, 1024], F32, name=f"qk{qb}_{h}_{kt}", tag="qk")
                for qs in range(2):
                    q0 = qb * 1024 + qs * 512
                    nc.tensor.matmul(
                        qk[:, qs * 512 : (qs + 1) * 512],
                        _r(KT[pt][r0 : r0 + DEP, kt * P : (kt + 1) * P]),
                        _r(QT[pt][r0 : r0 + DEP, q0 : q0 + 512]),
                        start=True,
                        stop=True,
                    )
                es = espool.tile([P, 1024], F32R, name=f"es{qb}_{h}_{kt}", tag="es")
                nc.scalar.activation(es, qk, EXP, scale=0.125)
                for qs in range(2):
                    nc.tensor.matmul(
                        pv[qs],
                        _r(V[kt][:, h * (DEP + 1) : (h + 1) * (DEP + 1)]),
                        _r(es[:, qs * 512 : (qs + 1) * 512]),
                        start=(kt == 0),
                        stop=(kt == NTT - 1),
                    )
            for qs in range(2):
                tq = qb * 2 + qs
                key = (pt, tq)
                if key not in OT_all:
                    OT_all[key] = xw.tile(
                        [P, 512], F32R, name=f"OT_{pt}_{tq}", tag="xt"
                    )
                rcp = rcpool.tile([1, 512], F32, name=f"rcp{qb}_{h}_{qs}", tag="rcp")
                nc.vector.reciprocal(rcp, pv[qs][DEP : DEP + 1, :])
                bct = bcpool.tile([DEP, 512], F32, name=f"bct{qb}_{h}_{qs}", tag="bct")
                nc.gpsimd.partition_broadcast(bct, rcp)
                nc.vector.tensor_mul(
                    OT_all[key][r0 : r0 + DEP, :], pv[qs][0:DEP, :], bct
                )

    # ---- dense ----------------------------------------------------------
    ds_ch = {}
    for pt in range(NPT):
        for nt in range(2):
            wt = wpool.tile(
                [P, PD], F32R, name=f"ds{pt}_{nt}", tag=f"w{8 + pt * 2 + nt}"
            )
            nc.sync.dma_start(
                out=wt,
                in_=_r(d["ds_t"][pt * P : (pt + 1) * P, nt * 512 : (nt + 1) * 512]),
            )
            ds_ch[(pt, nt)] = wt

    for tt in range(NTT):
        tq, tb = tt // 4, tt % 4
        for nt in range(2):
            pst = ps.tile([P, 512], F32, name=f"psd{tt}_{nt}", tag="proj")
            for pt in range(NPT):
                nc.tensor.matmul(
                    pst,
                    _r(OT_all[(pt, tq)][:, tb * P : (tb + 1) * P]),
                    _r(ds_ch[(pt, nt)]),
                    start=(pt == 0),
                    stop=(pt == NPT - 1),
                )
            osb = outpool.tile([P, 512], F32, name=f"osb{tt}_{nt}", tag="osb")
            nc.vector.tensor_copy(osb, pst)
            nc.sync.dma_start(
                out=d["out"][tt * P : (tt + 1) * P, nt * 512 : (nt + 1) * 512],
                in_=osb,
            )


def build_nc(with_bias=(False, False, False)):
    nc = bacc.Bacc("TRN2", target_bir_lowering=False, debug=False)
    d = {}
    for name, shape in (
        ("xq_t", [DM, S]),
        ("xk_t", [DM, S]),
        ("xv_t", [DM, S]),
        ("wq_t", [DM, PD]),
        ("wk_t", [DM, PD]),
        ("wv_t", [DM, PD]),
        ("ds_t", [PD, DM]),
        ("ones_c", [P, NH]),
    ):
        d[name] = nc.dram_tensor(name, shape, F32, kind="ExternalInput").ap()
    if any(with_bias):
        d["ones_r"] = nc.dram_tensor("ones_r", [1, 512], F32, kind="ExternalInput").ap()
    for flag, nm in zip(with_bias, ("bq", "bk", "bv")):
        if flag:
            d[nm] = nc.dram_tensor(nm, [1, PD], F32, kind="ExternalInput").ap()
    d["out"] = nc.dram_tensor("out", [S, DM], F32, kind="ExternalOutput").ap()

    with tile.TileContext(nc) as tc:
        with ExitStack() as ctx:
            _emit(nc, tc, ctx, d, with_bias)
    nc.compile()
    return nc


_CACHE = {}


def _get_nc(with_bias):
    if with_bias not in _CACHE:
        _CACHE[with_bias] = build_nc(with_bias)
    return _CACHE[with_bias]


def make_in_maps(query, key, value, wq_w, wq_b, wk_w, wk_b, wv_w, wv_b, dense_w):
    """Host-side sharding: 8 in_maps for cores (b, g)."""
    with_bias = (
        bool(np.any(wq_b)),
        bool(np.any(wk_b)),
        bool(np.any(wv_b)),
    )
    c = np.ascontiguousarray
    in_maps = []
    for core in range(8):
        b, g = core // 2, core % 2
        sl = slice(g * PD, (g + 1) * PD)
        m = {
            "xq_t": c(query[b].T),
            "xk_t": c(key[b].T),
            "xv_t": c(value[b].T),
            "wq_t": c(wq_w[sl].T),
            "wk_t": c(wk_w[sl].T),
            "wv_t": c(wv_w[sl].T),
            "ds_t": c(dense_w[:, sl].T),
            "ones_c": np.ones((P, NH), np.float32),
        }
        if any(with_bias):
            m["ones_r"] = np.ones((1, 512), np.float32)
        if with_bias[0]:
            m["bq"] = c(wq_b[sl][None, :])
        if with_bias[1]:
            m["bk"] = c(wk_b[sl][None, :])
        if with_bias[2]:
            m["bv"] = c(wv_b[sl][None, :])
        in_maps.append(m)
    return in_maps, with_bias


def kernel(
    query, key, value, wq_w, wq_b, wk_w, wk_b, wv_w, wv_b, dense_w, dense_b, **kw
):
    query = np.asarray(query, np.float32)
    key = np.asarray(key, np.float32)
    value = np.asarray(value, np.float32)
    in_maps, with_bias = make_in_maps(
        query, key, value,
        np.asarray(wq_w, np.float32), np.asarray(wq_b, np.float32),
        np.asarray(wk_w, np.float32), np.asarray(wk_b, np.float32),
        np.asarray(wv_w, np.float32), np.asarray(wv_b, np.float32),
        np.asarray(dense_w, np.float32),
    )
    nc = _get_nc(with_bias)
    res = run_bass_kernel_spmd(nc, in_maps, core_ids=list(range(8)))
    B = query.shape[0]
    out = np.empty((B, S, DM), np.float32)
    db = np.asarray(dense_b, np.float32)
    for b in range(B):
        out[b] = res.results[2 * b]["out"] + res.results[2 * b + 1]["out"] + db
    return out
